# revision 12
# baseline (speedup 1.0000x reference)
"""TRN2 Bass kernel for DHGCF-style GNN message passing (3-layer GCN with
edge/message dropout and row normalization), sharded across 8 NeuronCores.

Strategy (1D graph partitioning by destination row):
  - nodes (rows of emb / segment_sum outputs) are block-partitioned across the
    8 cores; small 64x64 weights replicated; edges partitioned by destination
    row so the scatter-add is core-local; an AllGather of the (dropped,
    un-normalized) hidden state runs between layers.
  - per core, edges are sorted by (dest row-tile of 128, source window of
    25000) and padded to 128-slot chunks.  The SpMM is:
        gather 128 source rows via SWDGE dma_gather (HBM fp32, 256B rows)
        S[e, r] = (iota_r == local_dest_row[e]) * a_val[e]      (one DVE op)
        side[r, :] += S^T @ gathered                            (PE, PSUM acc)
    then h = side @ W + b, message dropout, row-norm on DVE/ACT/PE.
  - dropout masks are the deterministic jax.random streams of the reference
    (key 42); they are reproduced host-side and shipped as {0, 1/(1-p)} float
    masks.  Dropped edges are removed host-side.
"""

import os
import sys

sys.path.insert(0, "/opt/trn_rl_repo")

import numpy as np

D = 64
NODE_DROP = 0.1
MESS_DROP = 0.1
EPS = 1e-12
TILE_R = 128


# ---------------------------------------------------------------- host side


def _compute_masks(n_nodes, n_edges):
    import jax

    with jax.default_device(jax.devices("cpu")[0]):
        drop_key = jax.random.key(42)
        k_node, k0, k1, k2 = jax.random.split(drop_key, 4)
        keep_e = np.asarray(
            jax.random.bernoulli(k_node, 1.0 - NODE_DROP, (n_edges,)))
        masks = [
            np.asarray(jax.random.bernoulli(k, 1.0 - MESS_DROP, (n_nodes, D)))
            for k in (k0, k1, k2)
        ]
    return keep_e, masks


class Plan:
    pass


def make_plan(rows, cols, vals, keep_e, n_nodes, ncores, win, gt):
    """Partition + sort edges, compute the (core-equalized) chunk structure
    and build the per-core device-layout index/value/row streams."""
    p = Plan()
    p.N = n_nodes
    p.NCORES = ncores
    p.RPC = n_nodes // ncores
    p.TILES = -(-p.RPC // TILE_R)
    p.WIN = win
    p.NWIN = -(-n_nodes // win)
    p.GT = gt
    p.NG = -(-p.TILES // gt)

    a_vals = np.where(keep_e, vals / (1.0 - NODE_DROP), 0.0).astype(np.float32)
    sel = keep_e
    r = rows[sel].astype(np.int64)
    c = cols[sel].astype(np.int64)
    av = a_vals[sel]

    core = r // p.RPC
    lrow = r - core * p.RPC
    til = lrow // TILE_R
    trw = lrow - til * TILE_R
    win_ = c // win
    wcol = c - win_ * win

    order = np.lexsort((win_, til, core))
    core, til, trw, win_, wcol, av = (
        core[order], til[order], trw[order], win_[order], wcol[order], av[order])

    key = (core * p.TILES + til) * p.NWIN + win_
    cnt = np.bincount(key, minlength=ncores * p.TILES * p.NWIN).reshape(
        ncores, p.TILES, p.NWIN)
    chunks_tw = -(-cnt.max(axis=0) // TILE_R)            # [TILES, NWIN]

    chunk_col = np.zeros((p.TILES, p.NWIN), dtype=np.int64)
    call_info = []                       # (g, w, chunk_col_base, ncols)
    group_base = np.zeros(p.NG, dtype=np.int64)
    off = 0
    for g in range(p.NG):
        t0, t1 = g * gt, min((g + 1) * gt, p.TILES)
        group_base[g] = off
        for w in range(p.NWIN):
            cb = off
            for t in range(t0, t1):
                chunk_col[t, w] = off
                off += chunks_tw[t, w]
            call_info.append((g, w, cb, off - cb))
    p.C = off
    p.S = off * TILE_R
    p.chunk_col = chunk_col
    p.chunks_tw = chunks_tw
    p.call_info = call_info
    p.group_base = group_base
    p.group_cols = np.array(
        [(group_base[g + 1] if g + 1 < p.NG else p.C) - group_base[g]
         for g in range(p.NG)])
    p.maxGC = int(p.group_cols.max())

    # slot of each (sorted) edge
    starts = np.flatnonzero(np.diff(key, prepend=-1))
    seg_start = np.repeat(starts, np.diff(np.append(starts, len(key))))
    rank = np.arange(len(key)) - seg_start
    slot = chunk_col[til, win_] * TILE_R + rank

    idx_slots = np.zeros((ncores, p.S), dtype=np.int16)
    av_slots = np.zeros((ncores, p.S), dtype=np.float32)
    trw_slots = np.zeros((ncores, p.S), dtype=np.float32)
    idx_slots[core, slot] = wcol.astype(np.int16)
    av_slots[core, slot] = av
    trw_slots[core, slot] = trw.astype(np.float32)

    # device layouts
    p.av_dev = np.ascontiguousarray(
        av_slots.reshape(ncores, p.C, TILE_R).transpose(0, 2, 1))
    p.trw_dev = np.ascontiguousarray(
        trw_slots.reshape(ncores, p.C, TILE_R).transpose(0, 2, 1))
    idx_dev = np.zeros((ncores, 128, p.S // 16), dtype=np.int16)
    for (_, _, cb, ncols) in call_info:
        if ncols == 0:
            continue
        s0, L = cb * TILE_R, ncols * TILE_R
        blk = idx_slots[:, s0:s0 + L].reshape(ncores, L // 16, 16)
        idx_dev[:, :, s0 // 16:(s0 + L) // 16] = np.tile(
            blk.transpose(0, 2, 1), (1, 8, 1))
    p.idx_dev = idx_dev

    p.tile_chunks = []
    for t in range(p.TILES):
        lst = []
        for w in range(p.NWIN):
            lst.extend(range(chunk_col[t, w], chunk_col[t, w] + chunks_tw[t, w]))
        p.tile_chunks.append(lst)
    return p


# ---------------------------------------------------------------- device side


def build_program(p):
    from concourse import bacc, bass, mybir, tile
    from contextlib import ExitStack

    f32 = mybir.dt.float32
    i16 = mybir.dt.int16
    Alu = mybir.AluOpType
    Act = mybir.ActivationFunctionType

    N, RPC, TILES, WIN, NWIN = p.N, p.RPC, p.TILES, p.WIN, p.NWIN
    NG, GT, C, S = p.NG, p.GT, p.C, p.S
    RPAD = TILES * TILE_R

    nc = bacc.Bacc("TRN2", target_bir_lowering=False, debug=False,
                   num_devices=p.NCORES)

    # consolidated fp32 resident block: one DMA -> one semaphore for all
    # resident consumers.  column layout:
    #   [0, C)            av
    #   [C, 2C)           trw
    #   [2C, 2C+128)      iota (row value = free index)
    #   [2C+128, 2C+256)  identity
    #   [2C+256, +192)    b replicated (3 layers x 64)
    #   [2C+448, +192)    W (rows 0..63; 3 layers x 64 cols)
    RES_COLS = 2 * C + 640
    emb_t = nc.dram_tensor("emb_tab", [N, D], f32, kind="ExternalInput")
    idx_t = nc.dram_tensor("idx_h", [128, S // 16], i16, kind="ExternalInput")
    res_t = nc.dram_tensor("res_h", [128, RES_COLS], f32, kind="ExternalInput")
    mask_t = nc.dram_tensor("mask_h", [3 * RPAD, D], f32, kind="ExternalInput")
    out_t = nc.dram_tensor("out_h", [RPAD, 3 * D], f32, kind="ExternalOutput")

    with ExitStack() as ctx:
        tc = ctx.enter_context(tile.TileContext(nc))
        singles = ctx.enter_context(tc.tile_pool(name="singles", bufs=1))
        gpool = ctx.enter_context(tc.tile_pool(name="gpool", bufs=2))
        mpool = ctx.enter_context(tc.tile_pool(name="mpool", bufs=3))
        work = ctx.enter_context(tc.tile_pool(name="work", bufs=3))
        spool = ctx.enter_context(tc.tile_pool(name="spool", bufs=4))
        ps_side = ctx.enter_context(
            tc.tile_pool(name="ps_side", bufs=2, space="PSUM"))
        ps_t = ctx.enter_context(tc.tile_pool(name="ps_t", bufs=2, space="PSUM"))
        ps_h = ctx.enter_context(tc.tile_pool(name="ps_h", bufs=2, space="PSUM"))
        dram = ctx.enter_context(tc.tile_pool(name="dram", bufs=1, space="DRAM"))

        # resident tiles
        idx_sb = singles.tile([128, S // 16], i16)
        nc.sync.dma_start(out=idx_sb[:], in_=idx_t[:, :])
        res_sb = singles.tile([128, RES_COLS], f32)
        nc.sync.dma_start(out=res_sb[:], in_=res_t[:, :])
        av_sb = res_sb[:, 0:C]
        trw_sb = res_sb[:, C:2 * C]
        iota_sb = res_sb[:, 2 * C:2 * C + 128]
        ident_sb = res_sb[:, 2 * C + 128:2 * C + 256]
        brep_sb = res_sb[:, 2 * C + 256:2 * C + 448]
        w_sb = res_sb[:64, 2 * C + 448:2 * C + 640]

        agin = [dram.tile([RPC, D], f32, name=f"agin{i}") for i in range(2)]
        agout = [dram.tile([N, D], f32, addr_space="Shared", name=f"agout{i}")
                 for i in range(2)]

        for layer in range(3):
            table = emb_t if layer == 0 else agout[layer - 1]
            for g in range(NG):
                gb = int(p.group_base[g])
                G = gpool.tile([128, p.maxGC, D], f32, tag="G", name=f"G_{layer}_{g}")
                for (gg, w, cb, ncols) in p.call_info:
                    if gg != g or ncols == 0:
                        continue
                    wsz = min(WIN, N - w * WIN)
                    # split large calls: SWDGE descriptor rings wedge on
                    # multi-thousand-descriptor single calls
                    MAXC = 8                      # <=1024 idxs per call
                    for c0 in range(0, ncols, MAXC):
                        cw = min(MAXC, ncols - c0)
                        L = cw * TILE_R
                        cbb = cb + c0
                        nc.gpsimd.dma_gather(
                            out_ap=G[:, cbb - gb:cbb - gb + cw, :],
                            in_ap=table[w * WIN:w * WIN + wsz, :],
                            idxs_ap=idx_sb[:, cbb * 8:cbb * 8 + L // 16],
                            num_idxs=L,
                            num_idxs_reg=L,
                            elem_size=D,
                        )
                for t in range(g * GT, min((g + 1) * GT, TILES)):
                    chunks = p.tile_chunks[t]
                    nch = len(chunks)
                    side_ps = ps_side.tile([128, D], f32, tag="side",
                                           name=f"sps_{layer}_{t}")
                    for k, ch in enumerate(chunks):
                        Smat = spool.tile([128, 128], f32, tag="S",
                                          name=f"S_{layer}_{t}_{k}")
                        nc.vector.tensor_scalar(
                            out=Smat[:], in0=iota_sb,
                            scalar1=trw_sb[:, ch:ch + 1],
                            scalar2=av_sb[:, ch:ch + 1],
                            op0=Alu.is_equal, op1=Alu.mult)
                        nc.tensor.matmul(
                            out=side_ps[:], lhsT=Smat[:],
                            rhs=G[:, ch - gb, :],
                            start=(k == 0), stop=(k == nch - 1))
                    side_sb = work.tile([128, D], f32, tag="side_sb",
                                        name=f"ssb_{layer}_{t}")
                    nc.vector.tensor_copy(out=side_sb[:], in_=side_ps[:])
                    sideT_ps = ps_t.tile([D, 128], f32, tag="sideT",
                                         name=f"stp_{layer}_{t}")
                    nc.tensor.transpose(sideT_ps[:], side_sb[:], ident_sb)
                    sideT_sb = work.tile([D, 128], f32, tag="sideT_sb",
                                         name=f"stb_{layer}_{t}")
                    nc.vector.tensor_copy(out=sideT_sb[:], in_=sideT_ps[:])
                    h_ps = ps_h.tile([128, D], f32, tag="h",
                                     name=f"hps_{layer}_{t}")
                    nc.tensor.matmul(
                        out=h_ps[:], lhsT=sideT_sb[:],
                        rhs=w_sb[:, layer * D:(layer + 1) * D],
                        start=True, stop=True)
                    msk = mpool.tile([128, D], f32, tag="mask",
                                     name=f"msk_{layer}_{t}")
                    nc.sync.dma_start(
                        out=msk[:],
                        in_=mask_t[layer * RPAD + t * TILE_R:
                                   layer * RPAD + (t + 1) * TILE_R, :])
                    h_sb = work.tile([128, D], f32, tag="h_sb",
                                     name=f"hsb_{layer}_{t}")
                    nc.vector.tensor_tensor(
                        out=h_sb[:], in0=h_ps[:],
                        in1=brep_sb[:, layer * D:(layer + 1) * D],
                        op=Alu.add)
                    nc.vector.tensor_tensor(
                        out=h_sb[:], in0=h_sb[:], in1=msk[:], op=Alu.mult)
                    nrows = min(TILE_R, RPC - t * TILE_R)
                    if layer < 2:
                        nc.sync.dma_start(
                            out=agin[layer][t * TILE_R:t * TILE_R + nrows, :],
                            in_=h_sb[:nrows, :])
                    sq = work.tile([128, D], f32, tag="sq",
                                   name=f"sq_{layer}_{t}")
                    ss = work.tile([128, 1], f32, tag="ss",
                                   name=f"ss_{layer}_{t}")
                    nc.scalar.activation(out=sq[:], in_=h_sb[:],
                                         func=Act.Square, accum_out=ss[:])
                    nv = work.tile([128, 1], f32, tag="nv",
                                   name=f"nv_{layer}_{t}")
                    nc.scalar.sqrt(nv[:], ss[:])
                    nv2 = work.tile([128, 1], f32, tag="nv2",
                                    name=f"nv2_{layer}_{t}")
                    nc.vector.tensor_scalar_max(nv2[:], nv[:], EPS)
                    ri = work.tile([128, 1], f32, tag="ri",
                                   name=f"ri_{layer}_{t}")
                    nc.vector.reciprocal(ri[:], nv2[:])
                    nrm = work.tile([128, D], f32, tag="nrm",
                                    name=f"nrm_{layer}_{t}")
                    nc.scalar.activation(out=nrm[:], in_=h_sb[:],
                                         func=Act.Copy, scale=ri[:, 0:1])
                    nc.sync.dma_start(
                        out=out_t[t * TILE_R:t * TILE_R + nrows,
                                  layer * D:(layer + 1) * D],
                        in_=nrm[:nrows, :])
            if layer < 2:
                nc.gpsimd.collective_compute(
                    "AllGather",
                    mybir.AluOpType.bypass,
                    replica_groups=[list(range(p.NCORES))],
                    ins=[agin[layer][:, :]],
                    outs=[agout[layer][:, :]],
                )
    nc.compile()
    return nc


def _run(inputs, n_nodes, n_edges, ncores=8, win=25000, gt=8, trace=False):
    rows = np.asarray(inputs["rows"])
    cols = np.asarray(inputs["cols"])
    vals = np.asarray(inputs["vals"])
    emb = np.asarray(inputs["emb"], dtype=np.float32)
    Ws = [np.asarray(inputs[f"W{i}"], dtype=np.float32) for i in range(3)]
    bs = [np.asarray(inputs[f"b{i}"], dtype=np.float32) for i in range(3)]

    keep_e, masks = _compute_masks(n_nodes, n_edges)
    p = make_plan(rows, cols, vals, keep_e, n_nodes, ncores, win, gt)

    RPAD = p.TILES * TILE_R
    C = p.C
    nc = build_program(p)

    from concourse import bass_utils

    # shared (replicated) part of the resident block
    res_fixed = np.zeros((128, 640), dtype=np.float32)
    res_fixed[:, 0:128] = np.tile(np.arange(128, dtype=np.float32), (128, 1))
    res_fixed[:, 128:256] = np.eye(128, dtype=np.float32)
    for l in range(3):
        res_fixed[:, 256 + l * D:256 + (l + 1) * D] = np.tile(bs[l], (128, 1))
        res_fixed[:D, 448 + l * D:448 + (l + 1) * D] = Ws[l]

    in_maps = []
    for cc in range(ncores):
        r0 = cc * p.RPC
        mh = np.zeros((3 * RPAD, D), dtype=np.float32)
        for l in range(3):
            mh[l * RPAD:l * RPAD + p.RPC] = np.where(
                masks[l][r0:r0 + p.RPC], 1.0 / (1.0 - MESS_DROP), 0.0)
        res_h = np.empty((128, 2 * C + 640), dtype=np.float32)
        res_h[:, 0:C] = p.av_dev[cc]
        res_h[:, C:2 * C] = p.trw_dev[cc]
        res_h[:, 2 * C:] = res_fixed
        in_maps.append({
            "emb_tab": emb,
            "idx_h": p.idx_dev[cc],
            "res_h": res_h,
            "mask_h": mh,
        })

    res = bass_utils.run_bass_kernel_spmd(
        nc, in_maps, core_ids=list(range(ncores)), trace=trace)

    out = np.empty((n_nodes, 4 * D), dtype=np.float32)
    out[:, :D] = emb
    for cc in range(ncores):
        out[cc * p.RPC:(cc + 1) * p.RPC, D:] = \
            res.results[cc]["out_h"][:p.RPC]
    _run.last_internals = (nc, in_maps, ncores, p)
    return out, res


def _bench(nc, in_maps, n_cores, iters=5):
    """Wall-clock the compiled NEFF executable with device-resident inputs
    (no donation, repeat executes).  Returns (min_seconds, all_times)."""
    import time as _time

    import jax
    from jax.experimental.shard_map import shard_map
    from jax.sharding import Mesh, NamedSharding, PartitionSpec

    from concourse import mybir
    from concourse.bass2jax import (_bass_exec_p, install_neuronx_cc_hook,
                                    partition_id_tensor)

    install_neuronx_cc_hook()
    partition_name = (nc.partition_id_tensor.name
                      if nc.partition_id_tensor else None)
    in_names, out_names, out_avals, zero_outs = [], [], [], []
    for alloc in nc.m.functions[0].allocations:
        if not isinstance(alloc, mybir.MemoryLocationSet):
            continue
        name = alloc.memorylocations[0].name
        if alloc.kind == "ExternalInput":
            if name != partition_name:
                in_names.append(name)
        elif alloc.kind == "ExternalOutput":
            shape = tuple(alloc.tensor_shape)
            dtype = mybir.dt.np(alloc.dtype)
            out_names.append(name)
            out_avals.append(jax.core.ShapedArray(shape, dtype))
            zero_outs.append(np.zeros(shape, dtype))
    n_params = len(in_names)
    all_in_names = list(in_names) + list(out_names)
    if partition_name is not None:
        all_in_names.append(partition_name)

    def _body(*args):
        operands = list(args)
        if partition_name is not None:
            operands.append(partition_id_tensor())
        return tuple(_bass_exec_p.bind(
            *operands,
            out_avals=tuple(out_avals),
            in_names=tuple(all_in_names),
            out_names=tuple(out_names),
            lowering_input_output_aliases=(),
            sim_require_finite=True,
            sim_require_nnan=True,
            nc=nc,
        ))

    devices = jax.devices()[:n_cores]
    mesh = Mesh(np.asarray(devices), ("core",))
    specs = (PartitionSpec("core"),) * (n_params + len(out_names))
    fn = jax.jit(shard_map(_body, mesh=mesh, in_specs=specs,
                           out_specs=(PartitionSpec("core"),) * len(out_names),
                           check_rep=False), keep_unused=True)
    sh = NamedSharding(mesh, PartitionSpec("core"))
    args = []
    for i, name in enumerate(in_names):
        cat = np.concatenate([np.asarray(m[name]) for m in in_maps], axis=0)
        args.append(jax.device_put(cat, sh))
    for z in zero_outs:
        cat = np.zeros((n_cores * z.shape[0], *z.shape[1:]), z.dtype)
        args.append(jax.device_put(cat, sh))
    out = fn(*args)
    jax.block_until_ready(out)
    times = []
    for _ in range(iters):
        t0 = _time.perf_counter()
        out = fn(*args)
        jax.block_until_ready(out)
        times.append(_time.perf_counter() - t0)
    return min(times), times


def kernel(rows, cols, vals, emb, W0, b0, W1, b1, W2, b2):
    inputs = dict(rows=rows, cols=cols, vals=vals, emb=emb,
                  W0=W0, b0=b0, W1=W1, b1=b1, W2=W2, b2=b2)
    out, _ = _run(inputs, n_nodes=emb.shape[0], n_edges=rows.shape[0],
                  trace=False)
    return out


# revision 14
# speedup vs baseline: 720.9924x; 720.9924x over previous
"""TRN2 Bass kernel for DHGCF-style GNN message passing (3-layer GCN with
edge/message dropout and row normalization), sharded across 8 NeuronCores.

Strategy (1D graph partitioning by destination row):
  - nodes (rows of emb / segment_sum outputs) are block-partitioned across the
    8 cores; small 64x64 weights replicated; edges partitioned by destination
    row so the scatter-add is core-local; an AllGather of the (dropped,
    un-normalized) hidden state runs between layers.
  - per core, edges are sorted by (dest row-tile of 128, source window of
    25000) and padded to 128-slot chunks.  The SpMM is:
        gather 128 source rows via SWDGE dma_gather (HBM fp32, 256B rows)
        S[e, r] = (iota_r == local_dest_row[e]) * a_val[e]      (one DVE op)
        side[r, :] += S^T @ gathered                            (PE, PSUM acc)
    then h = side @ W + b, message dropout, row-norm on DVE/ACT/PE.
  - dropout masks are the deterministic jax.random streams of the reference
    (key 42); they are reproduced host-side and shipped as {0, 1/(1-p)} float
    masks.  Dropped edges are removed host-side.
"""

import os
import sys

sys.path.insert(0, "/opt/trn_rl_repo")

import numpy as np

D = 64
NODE_DROP = 0.1
MESS_DROP = 0.1
EPS = 1e-12
TILE_R = 128


# ---------------------------------------------------------------- host side


def _compute_masks(n_nodes, n_edges):
    import jax

    with jax.default_device(jax.devices("cpu")[0]):
        drop_key = jax.random.key(42)
        k_node, k0, k1, k2 = jax.random.split(drop_key, 4)
        keep_e = np.asarray(
            jax.random.bernoulli(k_node, 1.0 - NODE_DROP, (n_edges,)))
        masks = [
            np.asarray(jax.random.bernoulli(k, 1.0 - MESS_DROP, (n_nodes, D)))
            for k in (k0, k1, k2)
        ]
    return keep_e, masks


class Plan:
    pass


def make_plan(rows, cols, vals, keep_e, n_nodes, ncores, win, gt):
    """Partition + sort edges, compute the (core-equalized) chunk structure
    and build the per-core device-layout index/value/row streams."""
    p = Plan()
    p.N = n_nodes
    p.NCORES = ncores
    p.RPC = n_nodes // ncores
    p.TILES = -(-p.RPC // TILE_R)
    p.WIN = win
    p.NWIN = -(-n_nodes // win)
    p.GT = gt
    p.NG = -(-p.TILES // gt)

    a_vals = np.where(keep_e, vals / (1.0 - NODE_DROP), 0.0).astype(np.float32)
    sel = keep_e
    r = rows[sel].astype(np.int64)
    c = cols[sel].astype(np.int64)
    av = a_vals[sel]

    core = r // p.RPC
    lrow = r - core * p.RPC
    til = lrow // TILE_R
    trw = lrow - til * TILE_R
    win_ = c // win
    wcol = c - win_ * win

    order = np.lexsort((win_, til, core))
    core, til, trw, win_, wcol, av = (
        core[order], til[order], trw[order], win_[order], wcol[order], av[order])

    key = (core * p.TILES + til) * p.NWIN + win_
    cnt = np.bincount(key, minlength=ncores * p.TILES * p.NWIN).reshape(
        ncores, p.TILES, p.NWIN)
    chunks_tw = -(-cnt.max(axis=0) // TILE_R)            # [TILES, NWIN]

    chunk_col = np.zeros((p.TILES, p.NWIN), dtype=np.int64)
    call_info = []                       # (g, w, chunk_col_base, ncols)
    group_base = np.zeros(p.NG, dtype=np.int64)
    off = 0
    for g in range(p.NG):
        t0, t1 = g * gt, min((g + 1) * gt, p.TILES)
        group_base[g] = off
        for w in range(p.NWIN):
            cb = off
            for t in range(t0, t1):
                chunk_col[t, w] = off
                off += chunks_tw[t, w]
            call_info.append((g, w, cb, off - cb))
    p.C = off
    p.S = off * TILE_R
    p.chunk_col = chunk_col
    p.chunks_tw = chunks_tw
    p.call_info = call_info
    p.group_base = group_base
    p.group_cols = np.array(
        [(group_base[g + 1] if g + 1 < p.NG else p.C) - group_base[g]
         for g in range(p.NG)])
    p.maxGC = int(p.group_cols.max())

    # slot of each (sorted) edge
    starts = np.flatnonzero(np.diff(key, prepend=-1))
    seg_start = np.repeat(starts, np.diff(np.append(starts, len(key))))
    rank = np.arange(len(key)) - seg_start
    slot = chunk_col[til, win_] * TILE_R + rank

    idx_slots = np.zeros((ncores, p.S), dtype=np.int16)
    av_slots = np.zeros((ncores, p.S), dtype=np.float32)
    trw_slots = np.zeros((ncores, p.S), dtype=np.float32)
    idx_slots[core, slot] = wcol.astype(np.int16)
    av_slots[core, slot] = av
    trw_slots[core, slot] = trw.astype(np.float32)

    # device layouts
    p.av_dev = np.ascontiguousarray(
        av_slots.reshape(ncores, p.C, TILE_R).transpose(0, 2, 1))
    p.trw_dev = np.ascontiguousarray(
        trw_slots.reshape(ncores, p.C, TILE_R).transpose(0, 2, 1))
    idx_dev = np.zeros((ncores, 128, p.S // 16), dtype=np.int16)
    for (_, _, cb, ncols) in call_info:
        if ncols == 0:
            continue
        s0, L = cb * TILE_R, ncols * TILE_R
        blk = idx_slots[:, s0:s0 + L].reshape(ncores, L // 16, 16)
        idx_dev[:, :, s0 // 16:(s0 + L) // 16] = np.tile(
            blk.transpose(0, 2, 1), (1, 8, 1))
    p.idx_dev = idx_dev

    p.tile_chunks = []
    for t in range(p.TILES):
        lst = []
        for w in range(p.NWIN):
            lst.extend(range(chunk_col[t, w], chunk_col[t, w] + chunks_tw[t, w]))
        p.tile_chunks.append(lst)
    return p


# ---------------------------------------------------------------- device side


def build_program(p):
    from concourse import bacc, bass, mybir, tile
    from contextlib import ExitStack

    f32 = mybir.dt.float32
    i16 = mybir.dt.int16
    Alu = mybir.AluOpType
    Act = mybir.ActivationFunctionType

    N, RPC, TILES, WIN, NWIN = p.N, p.RPC, p.TILES, p.WIN, p.NWIN
    NG, GT, C, S = p.NG, p.GT, p.C, p.S
    RPAD = TILES * TILE_R

    nc = bacc.Bacc("TRN2", target_bir_lowering=False, debug=False,
                   num_devices=p.NCORES)

    # consolidated fp32 resident block: one DMA -> one semaphore for all
    # resident consumers.  column layout:
    #   [0, C)            av
    #   [C, 2C)           trw
    #   [2C, 2C+128)      iota (row value = free index)
    #   [2C+128, 2C+256)  identity
    #   [2C+256, +192)    b replicated (3 layers x 64)
    #   [2C+448, +192)    W (rows 0..63; 3 layers x 64 cols)
    RES_COLS = 2 * C + 640
    emb_t = nc.dram_tensor("emb_tab", [N, D], f32, kind="ExternalInput")
    idx_t = nc.dram_tensor("idx_h", [128, S // 16], i16, kind="ExternalInput")
    res_t = nc.dram_tensor("res_h", [128, RES_COLS], f32, kind="ExternalInput")
    mask_t = nc.dram_tensor("mask_h", [3 * RPAD, D], f32, kind="ExternalInput")
    out_t = nc.dram_tensor("out_h", [RPAD, 3 * D], f32, kind="ExternalOutput")

    with ExitStack() as ctx:
        tc = ctx.enter_context(tile.TileContext(nc))
        singles = ctx.enter_context(tc.tile_pool(name="singles", bufs=1))
        gpool = ctx.enter_context(tc.tile_pool(name="gpool", bufs=2))
        mpool = ctx.enter_context(tc.tile_pool(name="mpool", bufs=3))
        work = ctx.enter_context(tc.tile_pool(name="work", bufs=3))
        spool = ctx.enter_context(tc.tile_pool(name="spool", bufs=4))
        ps_side = ctx.enter_context(
            tc.tile_pool(name="ps_side", bufs=2, space="PSUM"))
        ps_t = ctx.enter_context(tc.tile_pool(name="ps_t", bufs=2, space="PSUM"))
        ps_h = ctx.enter_context(tc.tile_pool(name="ps_h", bufs=2, space="PSUM"))
        dram = ctx.enter_context(tc.tile_pool(name="dram", bufs=1, space="DRAM"))

        # resident tiles
        idx_sb = singles.tile([128, S // 16], i16)
        nc.sync.dma_start(out=idx_sb[:], in_=idx_t[:, :])
        res_sb = singles.tile([128, RES_COLS], f32)
        nc.sync.dma_start(out=res_sb[:], in_=res_t[:, :])
        av_sb = res_sb[:, 0:C]
        trw_sb = res_sb[:, C:2 * C]
        iota_sb = res_sb[:, 2 * C:2 * C + 128]
        ident_sb = res_sb[:, 2 * C + 128:2 * C + 256]
        brep_sb = res_sb[:, 2 * C + 256:2 * C + 448]
        w_sb = res_sb[:64, 2 * C + 448:2 * C + 640]

        agin = [dram.tile([RPC, D], f32, name=f"agin{i}") for i in range(2)]
        agout = [dram.tile([N, D], f32, addr_space="Shared", name=f"agout{i}")
                 for i in range(2)]

        for layer in range(3):
            table = emb_t if layer == 0 else agout[layer - 1]
            for g in range(NG):
                gb = int(p.group_base[g])
                G = gpool.tile([128, p.maxGC, D], f32, tag="G", name=f"G_{layer}_{g}")
                for (gg, w, cb, ncols) in p.call_info:
                    if gg != g or ncols == 0:
                        continue
                    wsz = min(WIN, N - w * WIN)
                    # split large calls: SWDGE descriptor rings wedge on
                    # multi-thousand-descriptor single calls
                    MAXC = 8                      # <=1024 idxs per call
                    for c0 in range(0, ncols, MAXC):
                        cw = min(MAXC, ncols - c0)
                        L = cw * TILE_R
                        cbb = cb + c0
                        nc.gpsimd.dma_gather(
                            out_ap=G[:, cbb - gb:cbb - gb + cw, :],
                            in_ap=table[w * WIN:w * WIN + wsz, :],
                            idxs_ap=idx_sb[:, cbb * 8:cbb * 8 + L // 16],
                            num_idxs=L,
                            num_idxs_reg=L,
                            elem_size=D,
                        )
                for t in range(g * GT, min((g + 1) * GT, TILES)):
                    chunks = p.tile_chunks[t]
                    nch = len(chunks)
                    side_ps = ps_side.tile([128, D], f32, tag="side",
                                           name=f"sps_{layer}_{t}")
                    for k, ch in enumerate(chunks):
                        Smat = spool.tile([128, 128], f32, tag="S",
                                          name=f"S_{layer}_{t}_{k}")
                        nc.vector.tensor_scalar(
                            out=Smat[:], in0=iota_sb,
                            scalar1=trw_sb[:, ch:ch + 1],
                            scalar2=av_sb[:, ch:ch + 1],
                            op0=Alu.is_equal, op1=Alu.mult)
                        nc.tensor.matmul(
                            out=side_ps[:], lhsT=Smat[:],
                            rhs=G[:, ch - gb, :],
                            start=(k == 0), stop=(k == nch - 1))
                    side_sb = work.tile([128, D], f32, tag="side_sb",
                                        name=f"ssb_{layer}_{t}")
                    nc.vector.tensor_copy(out=side_sb[:], in_=side_ps[:])
                    sideT_ps = ps_t.tile([D, 128], f32, tag="sideT",
                                         name=f"stp_{layer}_{t}")
                    nc.tensor.transpose(sideT_ps[:], side_sb[:], ident_sb)
                    sideT_sb = work.tile([D, 128], f32, tag="sideT_sb",
                                         name=f"stb_{layer}_{t}")
                    nc.vector.tensor_copy(out=sideT_sb[:], in_=sideT_ps[:])
                    h_ps = ps_h.tile([128, D], f32, tag="h",
                                     name=f"hps_{layer}_{t}")
                    nc.tensor.matmul(
                        out=h_ps[:], lhsT=sideT_sb[:],
                        rhs=w_sb[:, layer * D:(layer + 1) * D],
                        start=True, stop=True)
                    msk = mpool.tile([128, D], f32, tag="mask",
                                     name=f"msk_{layer}_{t}")
                    nc.sync.dma_start(
                        out=msk[:],
                        in_=mask_t[layer * RPAD + t * TILE_R:
                                   layer * RPAD + (t + 1) * TILE_R, :])
                    h_sb = work.tile([128, D], f32, tag="h_sb",
                                     name=f"hsb_{layer}_{t}")
                    nc.vector.tensor_tensor(
                        out=h_sb[:], in0=h_ps[:],
                        in1=brep_sb[:, layer * D:(layer + 1) * D],
                        op=Alu.add)
                    nc.vector.tensor_tensor(
                        out=h_sb[:], in0=h_sb[:], in1=msk[:], op=Alu.mult)
                    nrows = min(TILE_R, RPC - t * TILE_R)
                    if layer < 2:
                        nc.sync.dma_start(
                            out=agin[layer][t * TILE_R:t * TILE_R + nrows, :],
                            in_=h_sb[:nrows, :])
                    sq = work.tile([128, D], f32, tag="sq",
                                   name=f"sq_{layer}_{t}")
                    ss = work.tile([128, 1], f32, tag="ss",
                                   name=f"ss_{layer}_{t}")
                    nc.scalar.activation(out=sq[:], in_=h_sb[:],
                                         func=Act.Square, accum_out=ss[:])
                    nv = work.tile([128, 1], f32, tag="nv",
                                   name=f"nv_{layer}_{t}")
                    nc.scalar.sqrt(nv[:], ss[:])
                    nv2 = work.tile([128, 1], f32, tag="nv2",
                                    name=f"nv2_{layer}_{t}")
                    nc.vector.tensor_scalar_max(nv2[:], nv[:], EPS)
                    ri = work.tile([128, 1], f32, tag="ri",
                                   name=f"ri_{layer}_{t}")
                    nc.vector.reciprocal(ri[:], nv2[:])
                    nrm = work.tile([128, D], f32, tag="nrm",
                                    name=f"nrm_{layer}_{t}")
                    nc.scalar.activation(out=nrm[:], in_=h_sb[:],
                                         func=Act.Copy, scale=ri[:, 0:1])
                    nc.sync.dma_start(
                        out=out_t[t * TILE_R:t * TILE_R + nrows,
                                  layer * D:(layer + 1) * D],
                        in_=nrm[:nrows, :])
            if layer < 2:
                nc.gpsimd.collective_compute(
                    "AllGather",
                    mybir.AluOpType.bypass,
                    replica_groups=[list(range(p.NCORES))],
                    ins=[agin[layer][:, :]],
                    outs=[agout[layer][:, :]],
                )
    nc.compile()
    return nc


def _run(inputs, n_nodes, n_edges, ncores=8, win=25000, gt=8, trace=False):
    rows = np.asarray(inputs["rows"])
    cols = np.asarray(inputs["cols"])
    vals = np.asarray(inputs["vals"])
    emb = np.asarray(inputs["emb"], dtype=np.float32)
    Ws = [np.asarray(inputs[f"W{i}"], dtype=np.float32) for i in range(3)]
    bs = [np.asarray(inputs[f"b{i}"], dtype=np.float32) for i in range(3)]

    keep_e, masks = _compute_masks(n_nodes, n_edges)
    p = make_plan(rows, cols, vals, keep_e, n_nodes, ncores, win, gt)

    RPAD = p.TILES * TILE_R
    C = p.C
    nc = build_program(p)

    from concourse import bass_utils

    # shared (replicated) part of the resident block
    res_fixed = np.zeros((128, 640), dtype=np.float32)
    res_fixed[:, 0:128] = np.tile(np.arange(128, dtype=np.float32), (128, 1))
    res_fixed[:, 128:256] = np.eye(128, dtype=np.float32)
    for l in range(3):
        res_fixed[:, 256 + l * D:256 + (l + 1) * D] = np.tile(bs[l], (128, 1))
        res_fixed[:D, 448 + l * D:448 + (l + 1) * D] = Ws[l]

    in_maps = []
    for cc in range(ncores):
        r0 = cc * p.RPC
        mh = np.zeros((3 * RPAD, D), dtype=np.float32)
        for l in range(3):
            mh[l * RPAD:l * RPAD + p.RPC] = np.where(
                masks[l][r0:r0 + p.RPC], 1.0 / (1.0 - MESS_DROP), 0.0)
        res_h = np.empty((128, 2 * C + 640), dtype=np.float32)
        res_h[:, 0:C] = p.av_dev[cc]
        res_h[:, C:2 * C] = p.trw_dev[cc]
        res_h[:, 2 * C:] = res_fixed
        in_maps.append({
            "emb_tab": emb,
            "idx_h": p.idx_dev[cc],
            "res_h": res_h,
            "mask_h": mh,
        })

    res = bass_utils.run_bass_kernel_spmd(
        nc, in_maps, core_ids=list(range(ncores)), trace=trace)

    out = np.empty((n_nodes, 4 * D), dtype=np.float32)
    out[:, :D] = emb
    for cc in range(ncores):
        out[cc * p.RPC:(cc + 1) * p.RPC, D:] = \
            res.results[cc]["out_h"][:p.RPC]
    _run.last_internals = (nc, in_maps, ncores, p)
    return out, res


def _bench(nc, in_maps, n_cores, iters=5, unroll=1):
    """Wall-clock the compiled NEFF executable with device-resident inputs
    (no donation, repeat executes).  Returns (min_seconds, all_times).
    With unroll>1, executes the NEFF `unroll` times per dispatch (ordered by
    jax effects) to amortize the ~80ms axon dispatch floor."""
    import time as _time

    import jax
    from jax.experimental.shard_map import shard_map
    from jax.sharding import Mesh, NamedSharding, PartitionSpec

    from concourse import mybir
    from concourse.bass2jax import (_bass_exec_p, install_neuronx_cc_hook,
                                    partition_id_tensor)

    install_neuronx_cc_hook()
    partition_name = (nc.partition_id_tensor.name
                      if nc.partition_id_tensor else None)
    in_names, out_names, out_avals, zero_outs = [], [], [], []
    for alloc in nc.m.functions[0].allocations:
        if not isinstance(alloc, mybir.MemoryLocationSet):
            continue
        name = alloc.memorylocations[0].name
        if alloc.kind == "ExternalInput":
            if name != partition_name:
                in_names.append(name)
        elif alloc.kind == "ExternalOutput":
            shape = tuple(alloc.tensor_shape)
            dtype = mybir.dt.np(alloc.dtype)
            out_names.append(name)
            out_avals.append(jax.core.ShapedArray(shape, dtype))
            zero_outs.append(np.zeros(shape, dtype))
    n_params = len(in_names)
    all_in_names = list(in_names) + list(out_names)
    if partition_name is not None:
        all_in_names.append(partition_name)

    def _body(*args):
        operands = list(args)
        if partition_name is not None:
            operands.append(partition_id_tensor())
        for _ in range(unroll):
            outs = _bass_exec_p.bind(
                *operands,
                out_avals=tuple(out_avals),
                in_names=tuple(all_in_names),
                out_names=tuple(out_names),
                lowering_input_output_aliases=(),
                sim_require_finite=True,
                sim_require_nnan=True,
                nc=nc,
            )
        return tuple(outs)

    devices = jax.devices()[:n_cores]
    mesh = Mesh(np.asarray(devices), ("core",))
    specs = (PartitionSpec("core"),) * (n_params + len(out_names))
    fn = jax.jit(shard_map(_body, mesh=mesh, in_specs=specs,
                           out_specs=(PartitionSpec("core"),) * len(out_names),
                           check_rep=False), keep_unused=True)
    sh = NamedSharding(mesh, PartitionSpec("core"))
    args = []
    for i, name in enumerate(in_names):
        cat = np.concatenate([np.asarray(m[name]) for m in in_maps], axis=0)
        args.append(jax.device_put(cat, sh))
    for z in zero_outs:
        cat = np.zeros((n_cores * z.shape[0], *z.shape[1:]), z.dtype)
        args.append(jax.device_put(cat, sh))
    out = fn(*args)
    jax.block_until_ready(out)
    times = []
    for _ in range(iters):
        t0 = _time.perf_counter()
        out = fn(*args)
        jax.block_until_ready(out)
        times.append(_time.perf_counter() - t0)
    return min(times), times


def kernel(rows, cols, vals, emb, W0, b0, W1, b1, W2, b2):
    inputs = dict(rows=rows, cols=cols, vals=vals, emb=emb,
                  W0=W0, b0=b0, W1=W1, b1=b1, W2=W2, b2=b2)
    out, _ = _run(inputs, n_nodes=emb.shape[0], n_edges=rows.shape[0],
                  trace=False)
    return out
